# revision 1
# baseline (speedup 1.0000x reference)
"""CategorySpecificLinear Trainium2 kernel.

out[t] = x[t] @ weight[category_id[t]] + bias[category_id[t]]

Strategy: expert-parallel over the 8 categories (C == n_cores == 8).
On the host we route tokens by category (the "all-to-all" happens at
sharding time since we receive full inputs), transpose each category's
token block to [D, T_pad] (the PE needs the contraction dim on
partitions and fp32 has no DMA-transpose), and hand core c:
    xT   [D, T_pad]   tokens of category c, zero-padded to T_pad
    w    [D, O]       weight[c]
    bias [1, O]       bias[c]
Each core computes out = xT.T @ w + bias with fp32r matmuls (full fp32
precision at 1 col/cycle for N>=256), then the host scatters the rows
back to token order.

Per-core HBM traffic ~8.7 MB (x 2.2 + w 4 + bias-bcast 0.5 + out 2.2);
fp32r MMs measure ~390 ns warm at N=512, so the 80-matmul stream is
~22 us and overlaps the ~24 us DMA stream. Measured on HW: 43.3 us
NEFF exec (incl. ~17 us fixed framework preamble/tail), rel err 1.5e-4.
"""

import contextlib
import ctypes
import os
import sys
import types

import numpy as np

sys.path.insert(0, "/opt/trn_rl_repo")


def _ensure_ntff_hook():
    """Provide antenv.axon_hooks if the image lacks it.

    concourse.bass_utils imports antenv.axon_hooks.get_axon_ntff_profile_hook
    when trace=True under axon; some agent images don't ship that module, in
    which case the boot's NTFF hook registration silently degrades and the
    import in bass_utils crashes. Recreate the slim ctypes hook here
    (mirrors trn_agent_boot.trn_boot._ntff_profile_via_ctypes).
    """
    try:
        import antenv.axon_hooks  # noqa: F401

        return
    except ImportError:
        pass

    so_path = "/opt/axon/libaxon_pjrt.so"
    hook = None
    if os.path.exists(so_path):
        lib = ctypes.CDLL(so_path)
        if hasattr(lib, "axon_start_nrt_profile"):
            lib.axon_start_nrt_profile.argtypes = [
                ctypes.POINTER(ctypes.c_int64),
                ctypes.c_size_t,
            ]
            lib.axon_start_nrt_profile.restype = ctypes.c_int64
            lib.axon_stop_nrt_profile.argtypes = [ctypes.c_char_p]
            lib.axon_stop_nrt_profile.restype = ctypes.c_int64

            @contextlib.contextmanager
            def hook(output_dir, device_ids):
                import jax

                jax.devices()
                if device_ids:
                    ids = (ctypes.c_int64 * len(device_ids))(*device_ids)
                    rc = lib.axon_start_nrt_profile(ids, len(device_ids))
                else:
                    rc = lib.axon_start_nrt_profile(None, 0)
                if rc != 0:
                    raise RuntimeError(f"axon_start_nrt_profile rc={rc}")
                try:
                    yield
                finally:
                    n = lib.axon_stop_nrt_profile(str(output_dir).encode())
                    if n <= 0:
                        print(
                            f"ntff profile: rc={n} writing {output_dir}",
                            file=sys.stderr,
                        )

    mod = types.ModuleType("antenv.axon_hooks")
    _state = {"hook": hook}
    mod.set_axon_ntff_profile_hook = lambda h: _state.__setitem__("hook", h)
    mod.get_axon_ntff_profile_hook = lambda: _state["hook"]
    sys.modules["antenv.axon_hooks"] = mod
    try:
        import antenv

        antenv.axon_hooks = mod
    except ImportError:
        pass


_ensure_ntff_hook()

import concourse.bass as bass
import concourse.bacc as bacc_mod
import concourse.mybir as mybir
import concourse.tile as tile
from concourse.bass import ts
from concourse.bass_utils import run_bass_kernel_spmd

N_CORES = 8
P = 128
N_TILE = 512  # one fp32 PSUM bank; also >=256 keeps fp32r at full rate

_nc_cache = {}
LAST_RESULTS = None  # BassKernelResults of the most recent run (for test.py)


def _build_nc(T_pad: int, D: int, O: int):
    KO = D // P
    NO = O // N_TILE
    mmdt = mybir.dt.float32r
    f32 = mybir.dt.float32

    # m-tiles: full 128-row tiles plus one remainder tile (multiple of 32)
    m_sizes = [P] * (T_pad // P)
    if T_pad % P:
        m_sizes.append(T_pad % P)
    MO = len(m_sizes)
    m_starts = [sum(m_sizes[:i]) for i in range(MO)]

    nc = bacc_mod.Bacc()
    xT = nc.dram_tensor("xT", [D, T_pad], mmdt, kind="ExternalInput")
    w = nc.dram_tensor("w", [D, O], mmdt, kind="ExternalInput")
    bias = nc.dram_tensor("bias", [1, O], f32, kind="ExternalInput")
    out = nc.dram_tensor("out", [T_pad, O], f32, kind="ExternalOutput")

    xT_t = xT[:, :].rearrange("(ko p) t -> p ko t", p=P)
    w_t = w[:, :].rearrange("(ko p) o -> p ko o", p=P)

    with tile.TileContext(nc) as tc:
        with (
            tc.tile_pool(name="resident", bufs=1) as rpool,
            tc.tile_pool(name="psum", bufs=7, space="PSUM") as psum_pool,
            tc.tile_pool(name="warmps", bufs=1, space="PSUM") as warm_pool,
            tc.tile_pool(name="obuf", bufs=6) as opool,
        ):
            # HAM warm-up: the PE is otherwise idle until the first k-slice
            # lands (~11 us); ~5 us of dummy matmuls gets the clock gate to
            # 8/8 so the real fp32r stream starts at warm speed (389 ns vs
            # 628 ns per MM). The dummy psum bank is never read.
            warm_sb = rpool.tile([P, 64], f32, tag="warm")
            nc.vector.memset(warm_sb[:], 0.0)
            warm_ps = warm_pool.tile([64, 64], f32, tag="wps")
            for i in range(24):
                nc.tensor.matmul(
                    warm_ps[:],
                    lhsT=warm_sb[:, :64],
                    rhs=warm_sb[:, :64],
                    start=True,
                    stop=True,
                )
            # Loads split over the two HWDGE engines (~650 ns serialized
            # issue cost each; one ~200 GB/s queue per engine). k-major so
            # wave A starts after the first k-slice pair, not the full 6 MB.
            # The 512 KB bias broadcast queues behind w(0..1,0) so it does
            # not delay the first matmul (DVE needs it much later).
            bias_sb = rpool.tile([P, O], f32, tag="bias")
            x_sb = []
            w_sb = {}
            for k in range(KO):
                xt = rpool.tile([P, T_pad], mmdt, tag=f"x{k}")
                nc.sync.dma_start(xt[:], xT_t[:, k, :])
                x_sb.append(xt)
                wt = rpool.tile([P, N_TILE], mmdt, tag=f"w{k}_0")
                nc.scalar.dma_start(wt[:], w_t[:, k, ts(0, N_TILE)])
                w_sb[(k, 0)] = wt
                if k == 1:
                    # broadcast from DRAM on the idle GpSimd queue so the
                    # 512 KB doesn't delay the w(:,0) stream on ACT
                    nc.gpsimd.dma_start(
                        bias_sb[:], bias[:, :].to_broadcast((P, O))
                    )
            for k in range(KO):
                for n in range(1, NO):
                    wt = rpool.tile([P, N_TILE], mmdt, tag=f"w{k}_{n}")
                    eng = nc.sync if k % 2 == 0 else nc.scalar
                    eng.dma_start(wt[:], w_t[:, k, ts(n, N_TILE)])
                    w_sb[(k, n)] = wt

            # One wave per n-tile: all MO psum groups accumulate in lockstep
            # over k, so the k-th step only needs x(k)/w(k,n) — PE starts
            # after the first ~600 KB instead of after the full 6 MB.
            for n in range(NO):
                pss = [
                    psum_pool.tile([m_sizes[m], N_TILE], f32, tag="ps", name=f"ps{n}_{m}")
                    for m in range(MO)
                ]
                for k in range(KO):
                    for m in range(MO):
                        nc.tensor.matmul(
                            pss[m][:],
                            lhsT=x_sb[k][:, m_starts[m] : m_starts[m] + m_sizes[m]],
                            rhs=w_sb[(k, n)][:],
                            start=(k == 0),
                            stop=(k == KO - 1),
                        )
                for m in range(MO):
                    ot = opool.tile([P, N_TILE], f32, tag="ot", name=f"ot{n}_{m}")
                    nc.vector.tensor_add(
                        ot[: m_sizes[m]], pss[m][:], bias_sb[: m_sizes[m], ts(n, N_TILE)]
                    )
                    nc.gpsimd.dma_start(
                        out[m_starts[m] : m_starts[m] + m_sizes[m], ts(n, N_TILE)],
                        ot[: m_sizes[m]],
                    )
    nc.finalize()
    return nc


def kernel(x, category_id, weight, bias):
    global LAST_RESULTS
    x = np.asarray(x)
    category_id = np.asarray(category_id)
    weight = np.ascontiguousarray(np.asarray(weight), dtype=np.float32)
    bias = np.ascontiguousarray(np.asarray(bias), dtype=np.float32)

    orig_shape = x.shape
    D = orig_shape[-1]
    C, _, O = weight.shape
    assert C == N_CORES and D % P == 0 and O % N_TILE == 0

    T = int(np.prod(orig_shape[:-1]))
    x_flat = np.ascontiguousarray(x.reshape(T, D), dtype=np.float32)
    cid = category_id.reshape(T).astype(np.int64)

    idx_per_c = [np.flatnonzero(cid == c) for c in range(C)]
    counts = [len(ix) for ix in idx_per_c]
    T_pad = max(32, -(-max(counts) // 32) * 32)  # multiple of 32 (PE col-group)

    key = (T_pad, D, O)
    if key not in _nc_cache:
        _nc_cache[key] = _build_nc(T_pad, D, O)
    nc = _nc_cache[key]

    in_maps = []
    for c in range(C):
        xcT = np.zeros((D, T_pad), dtype=np.float32)
        xcT[:, : counts[c]] = x_flat[idx_per_c[c]].T
        in_maps.append(
            {
                "xT": xcT,
                "w": weight[c],
                "bias": bias[c : c + 1],
            }
        )

    res = run_bass_kernel_spmd(nc, in_maps, list(range(N_CORES)))
    LAST_RESULTS = res

    out_flat = np.empty((T, O), dtype=np.float32)
    for c in range(C):
        out_flat[idx_per_c[c]] = res.results[c]["out"][: counts[c]]
    return out_flat.reshape(*orig_shape[:-1], O)



# revision 2
# speedup vs baseline: 1.1543x; 1.1543x over previous
"""CategorySpecificLinear Trainium2 kernel.

out[t] = x[t] @ weight[category_id[t]] + bias[category_id[t]]

Strategy: expert-parallel over the 8 categories (C == n_cores == 8).
Host routes tokens by category, transposes each category's token block
to [D, T_pad] and casts x/w to bf16 (fp32 accumulate in PSUM keeps the
rel err ~1e-3, far under the 2e-2 gate). Core c computes
    out = xT.T @ w + bias    (out in bf16, host casts back to fp32)

v2 vs the fp32r baseline (44.2us):
  - bf16 halves HBM traffic (3.4 MB/core vs 9.2) and matmul cost.
  - n=0 pass runs k-outer/m-inner so the PE consumes each k-slice as
    its DMA lands (x_k 0.27 MB + w_k 0.25 MB per slice); n=1 pass runs
    m-outer/k-inner so psum groups complete staggered and the bias-add
    + output DMA drain overlaps compute instead of piling up at the end.
  - out is written as one contiguous [m, 1024] bf16 DMA per m-tile.
  - fewer instructions => fewer tile semaphores => shorter framework
    wind-down epilogue (was ~9us of per-semaphore waits).
"""

import contextlib
import ctypes
import os
import sys
import types

import numpy as np
import ml_dtypes

sys.path.insert(0, "/opt/trn_rl_repo")

BF16 = np.dtype(ml_dtypes.bfloat16)


def _ensure_ntff_hook():
    """Provide antenv.axon_hooks if the image lacks it.

    concourse.bass_utils imports antenv.axon_hooks.get_axon_ntff_profile_hook
    when trace=True under axon; some agent images don't ship that module, in
    which case the boot's NTFF hook registration silently degrades and the
    import in bass_utils crashes. Recreate the slim ctypes hook here
    (mirrors trn_agent_boot.trn_boot._ntff_profile_via_ctypes).
    """
    try:
        import antenv.axon_hooks  # noqa: F401

        return
    except ImportError:
        pass

    so_path = "/opt/axon/libaxon_pjrt.so"
    hook = None
    if os.path.exists(so_path):
        lib = ctypes.CDLL(so_path)
        if hasattr(lib, "axon_start_nrt_profile"):
            lib.axon_start_nrt_profile.argtypes = [
                ctypes.POINTER(ctypes.c_int64),
                ctypes.c_size_t,
            ]
            lib.axon_start_nrt_profile.restype = ctypes.c_int64
            lib.axon_stop_nrt_profile.argtypes = [ctypes.c_char_p]
            lib.axon_stop_nrt_profile.restype = ctypes.c_int64

            @contextlib.contextmanager
            def hook(output_dir, device_ids):
                import jax

                jax.devices()
                if device_ids:
                    ids = (ctypes.c_int64 * len(device_ids))(*device_ids)
                    rc = lib.axon_start_nrt_profile(ids, len(device_ids))
                else:
                    rc = lib.axon_start_nrt_profile(None, 0)
                if rc != 0:
                    raise RuntimeError(f"axon_start_nrt_profile rc={rc}")
                try:
                    yield
                finally:
                    n = lib.axon_stop_nrt_profile(str(output_dir).encode())
                    if n <= 0:
                        print(
                            f"ntff profile: rc={n} writing {output_dir}",
                            file=sys.stderr,
                        )

    mod = types.ModuleType("antenv.axon_hooks")
    _state = {"hook": hook}
    mod.set_axon_ntff_profile_hook = lambda h: _state.__setitem__("hook", h)
    mod.get_axon_ntff_profile_hook = lambda: _state["hook"]
    sys.modules["antenv.axon_hooks"] = mod
    try:
        import antenv

        antenv.axon_hooks = mod
    except ImportError:
        pass


_ensure_ntff_hook()

import concourse.bass as bass
import concourse.bacc as bacc_mod
import concourse.mybir as mybir
import concourse.tile as tile
from concourse.bass import ts
from concourse.bass_utils import run_bass_kernel_spmd

N_CORES = 8
P = 128
N_TILE = 512  # one fp32 PSUM bank

_nc_cache = {}
LAST_RESULTS = None  # BassKernelResults of the most recent run (for test.py)


def _build_nc(T_pad: int, D: int, O: int):
    KO = D // P
    NO = O // N_TILE
    bf16 = mybir.dt.bfloat16
    f32 = mybir.dt.float32

    # m-tiles: full 128-row tiles plus one remainder tile (multiple of 32)
    m_sizes = [P] * (T_pad // P)
    if T_pad % P:
        m_sizes.append(T_pad % P)
    MO = len(m_sizes)
    m_starts = [sum(m_sizes[:i]) for i in range(MO)]

    nc = bacc_mod.Bacc()
    xT = nc.dram_tensor("xT", [D, T_pad], bf16, kind="ExternalInput")
    w = nc.dram_tensor("w", [D, O], bf16, kind="ExternalInput")
    bias = nc.dram_tensor("bias", [1, O], f32, kind="ExternalInput")
    out = nc.dram_tensor("out", [T_pad, O], bf16, kind="ExternalOutput")

    xT_t = xT[:, :].rearrange("(ko p) t -> p ko t", p=P)
    w_t = w[:, :].rearrange("(ko p) o -> p ko o", p=P)

    with tile.TileContext(nc) as tc:
        with (
            tc.tile_pool(name="resident", bufs=1) as rpool,
            tc.tile_pool(name="psum", bufs=8, space="PSUM") as psum_pool,
            tc.tile_pool(name="obuf", bufs=MO) as opool,
        ):
            # HAM warm-up: ~2.6 us of dummy matmuls lifts the PE clock
            # gate to 8/8 before the real stream starts. The dummy psum
            # tile is allocated from the main pool and never read, so
            # its bank is recycled once the pool wraps.
            warm_sb = rpool.tile([P, 64], f32, tag="warm")
            nc.vector.memset(warm_sb[:], 0.0)
            warm_ps = psum_pool.tile([64, 64], f32, tag="ps", name="warm_ps")
            for i in range(24):
                nc.tensor.matmul(
                    warm_ps[:],
                    lhsT=warm_sb[:, :64],
                    rhs=warm_sb[:, :64],
                    start=True,
                    stop=True,
                )
            # Input loads: one DMA per k-slice of x ([128, T_pad] bf16,
            # contiguous) and of w ([128, O] bf16, contiguous), alternated
            # across the two HWDGE queues so slice k lands ~k * 1.1 us in.
            # bias broadcast rides the idle gpsimd (SWDGE) queue.
            bias_sb = rpool.tile([P, O], f32, tag="bias")
            x_sb = []
            w_sb = []
            for k in range(KO):
                xt = rpool.tile([P, T_pad], bf16, tag=f"x{k}")
                wt = rpool.tile([P, O], bf16, tag=f"w{k}")
                if k % 2 == 0:
                    nc.sync.dma_start(xt[:], xT_t[:, k, :])
                    nc.scalar.dma_start(wt[:], w_t[:, k, :])
                else:
                    nc.scalar.dma_start(xt[:], xT_t[:, k, :])
                    nc.sync.dma_start(wt[:], w_t[:, k, :])
                x_sb.append(xt)
                w_sb.append(wt)
                if k == 1:
                    nc.gpsimd.dma_start(
                        bias_sb[:], bias[:, :].to_broadcast((P, O))
                    )

            obufs = [
                opool.tile([P, O], bf16, tag="ot", name=f"ot{m}")
                for m in range(MO)
            ]

            # Pass n=0: k-outer / m-inner. The PE touches k-slice k for
            # ~1.1 us (MO matmuls) which matches the DMA delivery rate,
            # so compute ramps with the loads instead of stalling on the
            # full 3.1 MB. All MO psum groups accumulate in lockstep.
            ps0 = [
                psum_pool.tile([m_sizes[m], N_TILE], f32, tag="ps", name=f"ps0_{m}")
                for m in range(MO)
            ]
            for k in range(KO):
                for m in range(MO):
                    nc.tensor.matmul(
                        ps0[m][:],
                        lhsT=x_sb[k][:, m_starts[m] : m_starts[m] + m_sizes[m]],
                        rhs=w_sb[k][:, ts(0, N_TILE)],
                        start=(k == 0),
                        stop=(k == KO - 1),
                    )
            for m in range(MO):
                nc.vector.tensor_add(
                    obufs[m][: m_sizes[m], ts(0, N_TILE)],
                    ps0[m][:],
                    bias_sb[: m_sizes[m], ts(0, N_TILE)],
                )

            # Pass n=1: m-outer / k-inner (inputs are all resident by
            # now). Each m-tile's psum group completes ~1.7 us apart, so
            # the bias-add and the single contiguous [m, O] output DMA
            # overlap the remaining matmuls. The last tile is the small
            # remainder, keeping the post-matmul tail ~1 us.
            for m in range(MO):
                ps = psum_pool.tile(
                    [m_sizes[m], N_TILE], f32, tag="ps", name=f"ps1_{m}"
                )
                for k in range(KO):
                    nc.tensor.matmul(
                        ps[:],
                        lhsT=x_sb[k][:, m_starts[m] : m_starts[m] + m_sizes[m]],
                        rhs=w_sb[k][:, ts(1, N_TILE)],
                        start=(k == 0),
                        stop=(k == KO - 1),
                    )
                nc.vector.tensor_add(
                    obufs[m][: m_sizes[m], ts(1, N_TILE)],
                    ps[:],
                    bias_sb[: m_sizes[m], ts(1, N_TILE)],
                )
                eng = nc.sync if m % 2 == 0 else nc.scalar
                eng.dma_start(
                    out[m_starts[m] : m_starts[m] + m_sizes[m], :],
                    obufs[m][: m_sizes[m], :],
                )
    nc.finalize()
    return nc


def kernel(x, category_id, weight, bias):
    global LAST_RESULTS
    x = np.asarray(x)
    category_id = np.asarray(category_id)
    weight = np.asarray(weight, dtype=np.float32)
    bias = np.ascontiguousarray(np.asarray(bias), dtype=np.float32)

    orig_shape = x.shape
    D = orig_shape[-1]
    C, _, O = weight.shape
    assert C == N_CORES and D % P == 0 and O % N_TILE == 0

    T = int(np.prod(orig_shape[:-1]))
    x_flat = np.ascontiguousarray(x.reshape(T, D), dtype=np.float32)
    cid = category_id.reshape(T).astype(np.int64)

    idx_per_c = [np.flatnonzero(cid == c) for c in range(C)]
    counts = [len(ix) for ix in idx_per_c]
    T_pad = max(32, -(-max(counts) // 32) * 32)  # multiple of 32 (PE col-group)

    key = (T_pad, D, O)
    if key not in _nc_cache:
        _nc_cache[key] = _build_nc(T_pad, D, O)
    nc = _nc_cache[key]

    w_bf16 = weight.astype(BF16)
    in_maps = []
    for c in range(C):
        xcT = np.zeros((D, T_pad), dtype=BF16)
        xcT[:, : counts[c]] = x_flat[idx_per_c[c]].T.astype(BF16)
        in_maps.append(
            {
                "xT": xcT,
                "w": w_bf16[c],
                "bias": bias[c : c + 1],
            }
        )

    res = run_bass_kernel_spmd(nc, in_maps, list(range(N_CORES)))
    LAST_RESULTS = res

    out_flat = np.empty((T, O), dtype=np.float32)
    for c in range(C):
        out_flat[idx_per_c[c]] = res.results[c]["out"][: counts[c]].astype(
            np.float32
        )
    return out_flat.reshape(*orig_shape[:-1], O)
